# revision 14
# baseline (speedup 1.0000x reference)
"""Expert-parallel BaseLayer MoE kernel for 8 TRN2 NeuronCores.

Strategy: all routing, layernorm, gating and the residual live on the host;
the device runs only the two FFN matmuls.  Work is split expert-parallel with
2-way f-sharding for load balance: each core owns two (expert, f-half) slots —
block 1 holds a half of one of the 4 largest experts, block 2 a half of one of
the 4 smallest — so the padded capacity is max_count + 5th_count instead of
2*max_count.  m1 runs in bf16; m2 runs in fp8-e4m3 DoubleRow (two k-tiles per
instruction at 2x rate), with the relu output quantized to fp8 by the scalar
engine and the dequant folded into the output-copy epilogue.  The host sums
the two f-half partials per expert and applies the sigmoid gate + residual.
"""

import functools
import sys

import numpy as np

for _p in ("/opt/trn_rl_repo", "/opt/pypackages"):
    if _p not in sys.path:
        sys.path.append(_p)

import ml_dtypes  # noqa: E402

import concourse.bass as bass  # noqa: E402
import concourse.mybir as mybir  # noqa: E402
import concourse.tile as tile  # noqa: E402
from concourse import bacc  # noqa: E402
from concourse import bass_utils  # noqa: E402


def _ensure_axon_hooks():
    """bass_utils' trace path imports antenv.axon_hooks, which some agent
    images lack; synthesize it (with the real ctypes NTFF hook when
    available) so tracing degrades gracefully instead of crashing."""
    try:
        import antenv.axon_hooks  # noqa: F401
        return
    except ImportError:
        pass
    import types

    import antenv

    hooks = types.ModuleType("antenv.axon_hooks")
    hooks._hook = None
    hooks.set_axon_ntff_profile_hook = lambda h: setattr(hooks, "_hook", h)
    hooks.get_axon_ntff_profile_hook = lambda: hooks._hook
    sys.modules["antenv.axon_hooks"] = hooks
    antenv.axon_hooks = hooks
    try:
        from trn_agent_boot.trn_boot import _ntff_profile_via_ctypes

        hooks._hook = _ntff_profile_via_ctypes("/opt/axon/libaxon_pjrt.so")
    except Exception:
        pass


_ensure_axon_hooks()

E = 8
D = 1024
F = 4096
FH = F // 2      # f-half per slot
KD = D // 128    # 8 k-tiles over d
JF = FH // 128   # 16 f-tiles per half (m1 output tiles)
KJ2 = FH // 128  # 16 k2-tiles per half (m2 contraction)
EPS = 1e-5

USE_FP8_M2 = True
N_FP8 = 4        # leading f-tiles per half computed in fp8 DoubleRow m1
SZ = 32.0        # z (relu output) fp8 scale
SW2 = 2048.0     # w2 fp8 scale
SX = 32.0        # xhat fp8 scale (m1 fp8 tiles)
SW1 = 1024.0     # w1 fp8 scale (m1 fp8 tiles)

F32 = mybir.dt.float32
BF16 = mybir.dt.bfloat16
FP8 = mybir.dt.float8e4
AF = mybir.ActivationFunctionType
DR = mybir.MatmulPerfMode.DoubleRow


@functools.lru_cache(maxsize=4)
def _build(b1_cap, b2_cap, fp8_m2, nq):
    k_total = b1_cap + b2_cap
    z_dt = FP8 if fp8_m2 else BF16
    w2_dt = FP8 if fp8_m2 else BF16
    jbf = JF - nq  # bf16 f-tiles per half

    nc = bacc.Bacc("TRN2", target_bir_lowering=False, debug=False, num_devices=E)

    xh = nc.declare_dram_parameter("xh", [KD, 128, k_total], BF16, isOutput=False)
    if nq:
        xh8 = nc.declare_dram_parameter(
            "xh8", [KD // 2, 128, 2, k_total], FP8, isOutput=False
        )
        w1q = nc.declare_dram_parameter(
            "w1q", [2 * nq, 128, KD, 128], FP8, isOutput=False
        )
    w1t = nc.declare_dram_parameter("w1t", [2 * jbf, 128, KD, 128], BF16, isOutput=False)
    b1c = nc.declare_dram_parameter("b1c", [128, 2 * JF], F32, isOutput=False)
    w2t = nc.declare_dram_parameter("w2t", [2 * KD, 128, KJ2, 128], w2_dt, isOutput=False)
    out1 = nc.declare_dram_parameter("out1", [128, KD, b1_cap], BF16, isOutput=True)
    out2 = nc.declare_dram_parameter("out2", [128, KD, b2_cap], BF16, isOutput=True)

    blocks = [(bass.ds(0, b1_cap), b1_cap), (bass.ds(b1_cap, b2_cap), b2_cap)]
    outs_d = [out1, out2]

    with tile.TileContext(nc) as tc:
        with (
            tc.tile_pool(name="const", bufs=1) as constp,
            tc.tile_pool(name="xhp", bufs=1) as xhp,
            tc.tile_pool(name="zp", bufs=1) as zp,
            tc.tile_pool(name="w1p", bufs=8) as w1p,
            tc.tile_pool(name="w1qp", bufs=8) as w1qp,
            tc.tile_pool(name="w2p", bufs=2 * KD) as w2p,
            tc.tile_pool(name="outp", bufs=1) as outp,
            tc.tile_pool(name="ps_z", bufs=4, space=bass.MemorySpace.PSUM) as psz,
            tc.tile_pool(name="ps_y", bufs=3, space=bass.MemorySpace.PSUM) as psy,
        ):
            # --- input DMAs: fp8 x first (small, feeds the fp8 m1 tiles that
            # run during the DMA catch-up window), then bf16 x per k-tile ---
            xk = []
            for k in range(KD):
                t = xhp.tile([128, k_total], BF16, tag=f"xk{k}", name=f"xk{k}")
                nc.scalar.dma_start(out=t[:, : b1_cap], in_=xh[k][:, : b1_cap])
                xk.append(t)
            for k in range(KD):
                nc.scalar.dma_start(
                    out=xk[k][:, b1_cap:], in_=xh[k][:, b1_cap:]
                )

            b1_sb = constp.tile([128, 2 * JF], F32, tag="b1")
            nc.gpsimd.dma_start(out=b1_sb[:], in_=b1c[:])
            xq = []
            for p in range(KD // 2 if nq else 0):
                t = xhp.tile([128, 2, k_total], FP8, tag=f"xq{p}", name=f"xq{p}")
                nc.gpsimd.dma_start(out=t[:], in_=xh8[p])
                xq.append(t)

            z_tiles = [
                zp.tile([128, JF, bc], z_dt, tag=f"z{bi}", name=f"z{bi}")
                for bi, (_, bc) in enumerate(blocks)
            ]
            ost = [
                outp.tile([128, KD, bc], BF16, tag=f"o{bi}", name=f"o{bi}")
                for bi, (_, bc) in enumerate(blocks)
            ]

            # --- m1: z = relu(SZ*(w1^T xhat) + SZ*b1), quantized to z_dt;
            # first nq f-tiles per half in fp8 DoubleRow, rest bf16 ---
            for bi, (csl, bc) in enumerate(blocks):
                for j in range(JF):
                    pz = psz.tile([128, bc], F32, tag="z")
                    if j >= jbf:
                        w1sb = w1qp.tile([128, KD, 128], FP8, tag="w1q")
                        nc.sync.dma_start(out=w1sb[:], in_=w1q[bi * nq + (j - jbf)])
                        for p in range(KD // 2):
                            nc.tensor.matmul(
                                pz[:], w1sb[:, 2 * p : 2 * p + 2, :],
                                xq[p][:, :, csl],
                                start=(p == 0), stop=(p == KD // 2 - 1),
                                perf_mode=DR,
                            )
                        act_scale = SZ / (SX * SW1)
                    else:
                        w1sb = w1p.tile([128, KD, 128], BF16, tag="w1")
                        nc.sync.dma_start(out=w1sb[:], in_=w1t[bi * jbf + j])
                        for k in range(KD):
                            nc.tensor.matmul(
                                pz[:], w1sb[:, k, :], xk[k][:, csl],
                                start=(k == 0), stop=(k == KD - 1),
                            )
                        act_scale = SZ if fp8_m2 else 1.0
                    nc.scalar.activation(
                        z_tiles[bi][:, j, :], pz[:], AF.Relu,
                        bias=b1_sb[:, bi * JF + j : bi * JF + j + 1],
                        scale=act_scale,
                    )

            # --- w2 tiles: queued on the sync ring behind all w1 traffic ---
            w2_tiles = []
            for s in range(2 * KD):
                w2sb = w2p.tile([128, KJ2, 128], w2_dt, tag="w2")
                nc.sync.dma_start(out=w2sb[:], in_=w2t[s])
                w2_tiles.append(w2sb)

            # --- m2: y = (z @ w2) / (SZ*SW2), bf16 out staged per block ---
            dq = 1.0 / (SZ * SW2) if fp8_m2 else 1.0
            for bi, (csl, bc) in enumerate(blocks):
                z_sb = z_tiles[bi]
                for i in range(KD):
                    w2sb = w2_tiles[bi * KD + i]
                    py = psy.tile([128, bc], F32, tag="y")
                    if fp8_m2:
                        for q in range(KJ2 // 2):
                            nc.tensor.matmul(
                                py[:], w2sb[:, 2 * q : 2 * q + 2, :],
                                z_sb[:, 2 * q : 2 * q + 2, :],
                                start=(q == 0), stop=(q == KJ2 // 2 - 1),
                                perf_mode=DR,
                            )
                    else:
                        for q in range(KJ2):
                            nc.tensor.matmul(
                                py[:], w2sb[:, q, :], z_sb[:, q, :],
                                start=(q == 0), stop=(q == KJ2 - 1),
                            )
                    nc.scalar.activation(
                        ost[bi][:, i, :], py[:], AF.Copy, bias=0.0, scale=dq
                    )
                    nc.gpsimd.dma_start(
                        out=outs_d[bi][:, i : i + 1, :],
                        in_=ost[bi][:, i : i + 1, :],
                    )

    nc.compile()
    return nc


def _pad32(n):
    return int(max(32, ((n + 3) // 4) * 4))


def kernel(x, centroids, w1, b1, w2, b2, gamma, beta):
    x = np.ascontiguousarray(np.asarray(x, dtype=np.float32))
    centroids = np.asarray(centroids, dtype=np.float32)
    w1 = np.asarray(w1, dtype=np.float32)
    b1 = np.asarray(b1, dtype=np.float32)
    w2 = np.asarray(w2, dtype=np.float32)
    b2 = np.asarray(b2, dtype=np.float32)
    gamma = np.asarray(gamma, dtype=np.float32)
    beta = np.asarray(beta, dtype=np.float32)

    orig_shape = x.shape
    feats = x.reshape(-1, D)

    # --- host: routing + layernorm + gate (same math as the reference) ---
    aff = feats @ centroids.T
    eid = np.argmax(aff, axis=1)
    idxs = [np.nonzero(eid == e)[0] for e in range(E)]
    counts = np.array([len(ix) for ix in idxs])

    mu = feats.mean(-1, keepdims=True)
    var = feats.var(-1, keepdims=True)
    xhat = (feats - mu) / np.sqrt(var + EPS)

    # slot assignment: block 1 = halves of the 4 largest experts, block 2 =
    # halves of the 4 smallest; core c gets (ranked[c//2], half c%2) and
    # (ranked[4+c//2], half c%2).
    ranked = np.argsort(-counts, kind="stable")
    b1_cap = _pad32(counts[ranked[0]])
    b2_cap = _pad32(counts[ranked[4]])
    assert b1_cap <= 512 and b2_cap <= 512, (b1_cap, b2_cap)
    k_total = b1_cap + b2_cap

    nq = N_FP8 if USE_FP8_M2 else 0
    jbf = JF - nq
    nc = _build(b1_cap, b2_cap, USE_FP8_M2, nq)

    f8 = ml_dtypes.float8_e4m3

    def q8(a):
        return np.clip(a, -240.0, 240.0).astype(f8)

    in_maps = []
    slot_info = []  # per core: [(expert, half, offset, count), ...]
    for c in range(E):
        slots = [
            (int(ranked[c // 2]), c % 2, 0, b1_cap),
            (int(ranked[4 + c // 2]), c % 2, b1_cap, b2_cap),
        ]
        xh_full = np.zeros((D, k_total), dtype=np.float32)
        w1_tiles = np.empty((2 * jbf, 128, KD, 128), dtype=ml_dtypes.bfloat16)
        w1q_tiles = np.empty((2 * nq, 128, KD, 128), dtype=f8)
        b1_cols = np.zeros((128, 2 * JF), dtype=np.float32)
        w2_tiles = np.empty((2 * KD, 128, KJ2, 128), dtype=f8 if USE_FP8_M2 else ml_dtypes.bfloat16)
        info = []
        for bi, (e, h, off, cap) in enumerate(slots):
            n_e = counts[e]
            xh_full[:, off : off + n_e] = xhat[idxs[e]].T
            hsl = slice(h * FH, (h + 1) * FH)
            w1e = (gamma[e][:, None] * w1[e])[:, hsl]          # [D, FH]
            b1e = (b1[e] + beta[e] @ w1[e])[hsl]               # [FH]
            w1_t = w1e.reshape(KD, 128, JF, 128).transpose(2, 1, 0, 3)  # [JF,128,KD,128]
            w1q_tiles[bi * nq : (bi + 1) * nq] = q8(w1_t[jbf:] * SW1)
            w1_tiles[bi * jbf : (bi + 1) * jbf] = w1_t[:jbf].astype(ml_dtypes.bfloat16)
            b1_cols[:, bi * JF : (bi + 1) * JF] = (
                (SZ if USE_FP8_M2 else 1.0) * b1e
            ).reshape(JF, 128).T
            w2e = w2[e][hsl, :]                                # [FH, D]
            if USE_FP8_M2:
                w2q = q8(w2e * SW2)
            else:
                w2q = w2e.astype(ml_dtypes.bfloat16)
            w2_tiles[bi * KD : (bi + 1) * KD] = (
                w2q.reshape(KJ2, 128, KD, 128).transpose(2, 1, 0, 3)
            )
            info.append((e, h, off, n_e))
        xh_kt = xh_full.reshape(KD, 128, k_total)
        xh_t = np.ascontiguousarray(xh_kt).astype(ml_dtypes.bfloat16)
        im = dict(xh=xh_t, w1t=w1_tiles, b1c=b1_cols, w2t=w2_tiles)
        if nq:
            im["xh8"] = np.ascontiguousarray(
                q8(xh_kt * SX).reshape(KD // 2, 2, 128, k_total).transpose(0, 2, 1, 3)
            )
            im["w1q"] = w1q_tiles
        in_maps.append(im)
        slot_info.append(info)

    res = bass_utils.run_bass_kernel_spmd(nc, in_maps, core_ids=list(range(E)))
    kernel._last_res = res

    # --- host: sum f-half partials, gate, residual, scatter ---
    y_sum = [None] * E
    for c in range(E):
        arrs = [
            np.asarray(res.results[c]["out1"]).astype(np.float32),
            np.asarray(res.results[c]["out2"]).astype(np.float32),
        ]  # each [128, KD, B] -> [D, B]
        for bi, (e, h, off, n_e) in enumerate(slot_info[c]):
            a = arrs[bi].transpose(1, 0, 2).reshape(D, -1)
            part = a[:, :n_e].T                                # [n_e, D]
            y_sum[e] = part if y_sum[e] is None else y_sum[e] + part

    out = np.empty_like(feats)
    for e in range(E):
        ix = idxs[e]
        if len(ix) == 0:
            continue
        al = 1.0 / (1.0 + np.exp(-aff[ix, e]))[:, None]
        out[ix] = feats[ix] + al * (y_sum[e] + b2[e])
    return out.reshape(orig_shape)


# revision 15
# speedup vs baseline: 1.0225x; 1.0225x over previous
"""Expert-parallel BaseLayer MoE kernel for 8 TRN2 NeuronCores.

Strategy: all routing, layernorm, gating and the residual live on the host;
the device runs only the two FFN matmuls.  Work is split expert-parallel with
2-way f-sharding for load balance: each core owns two (expert, f-half) slots —
block 1 holds a half of one of the 4 largest experts, block 2 a half of one of
the 4 smallest — so the padded capacity is max_count + 5th_count instead of
2*max_count.  m1 runs in bf16; m2 runs in fp8-e4m3 DoubleRow (two k-tiles per
instruction at 2x rate), with the relu output quantized to fp8 by the scalar
engine and the dequant folded into the output-copy epilogue.  The host sums
the two f-half partials per expert and applies the sigmoid gate + residual.
"""

import functools
import sys

import numpy as np

for _p in ("/opt/trn_rl_repo", "/opt/pypackages"):
    if _p not in sys.path:
        sys.path.append(_p)

import ml_dtypes  # noqa: E402

import concourse.bass as bass  # noqa: E402
import concourse.mybir as mybir  # noqa: E402
import concourse.tile as tile  # noqa: E402
from concourse import bacc  # noqa: E402
from concourse import bass_utils  # noqa: E402


def _ensure_axon_hooks():
    """bass_utils' trace path imports antenv.axon_hooks, which some agent
    images lack; synthesize it (with the real ctypes NTFF hook when
    available) so tracing degrades gracefully instead of crashing."""
    try:
        import antenv.axon_hooks  # noqa: F401
        return
    except ImportError:
        pass
    import types

    import antenv

    hooks = types.ModuleType("antenv.axon_hooks")
    hooks._hook = None
    hooks.set_axon_ntff_profile_hook = lambda h: setattr(hooks, "_hook", h)
    hooks.get_axon_ntff_profile_hook = lambda: hooks._hook
    sys.modules["antenv.axon_hooks"] = hooks
    antenv.axon_hooks = hooks
    try:
        from trn_agent_boot.trn_boot import _ntff_profile_via_ctypes

        hooks._hook = _ntff_profile_via_ctypes("/opt/axon/libaxon_pjrt.so")
    except Exception:
        pass


_ensure_axon_hooks()

E = 8
D = 1024
F = 4096
FH = F // 2      # f-half per slot
KD = D // 128    # 8 k-tiles over d
JF = FH // 128   # 16 f-tiles per half (m1 output tiles)
KJ2 = FH // 128  # 16 k2-tiles per half (m2 contraction)
EPS = 1e-5

USE_FP8_M2 = True
N_FP8 = 4        # leading f-tiles per half computed in fp8 DoubleRow m1
SZ = 32.0        # z (relu output) fp8 scale
SW2 = 2048.0     # w2 fp8 scale
SX = 32.0        # xhat fp8 scale (m1 fp8 tiles)
SW1 = 1024.0     # w1 fp8 scale (m1 fp8 tiles)

F32 = mybir.dt.float32
BF16 = mybir.dt.bfloat16
FP8 = mybir.dt.float8e4
AF = mybir.ActivationFunctionType
DR = mybir.MatmulPerfMode.DoubleRow


@functools.lru_cache(maxsize=4)
def _build(b1_cap, b2_cap, fp8_m2, nq):
    k_total = b1_cap + b2_cap
    z_dt = FP8 if fp8_m2 else BF16
    w2_dt = FP8 if fp8_m2 else BF16
    jbf = JF - nq  # bf16 f-tiles per half

    nc = bacc.Bacc("TRN2", target_bir_lowering=False, debug=False, num_devices=E)

    xh = nc.declare_dram_parameter("xh", [KD, 128, k_total], BF16, isOutput=False)
    if nq:
        xh8 = nc.declare_dram_parameter(
            "xh8", [KD // 2, 128, 2, k_total], FP8, isOutput=False
        )
        w1q = nc.declare_dram_parameter(
            "w1q", [2 * nq, 128, KD, 128], FP8, isOutput=False
        )
    w1t = nc.declare_dram_parameter("w1t", [2 * jbf, 128, KD, 128], BF16, isOutput=False)
    b1c = nc.declare_dram_parameter("b1c", [128, 2 * JF], F32, isOutput=False)
    w2t = nc.declare_dram_parameter("w2t", [2 * KD, 128, KJ2, 128], w2_dt, isOutput=False)
    out1 = nc.declare_dram_parameter("out1", [128, KD, b1_cap], BF16, isOutput=True)
    out2 = nc.declare_dram_parameter("out2", [128, KD, b2_cap], BF16, isOutput=True)

    blocks = [(bass.ds(0, b1_cap), b1_cap), (bass.ds(b1_cap, b2_cap), b2_cap)]
    outs_d = [out1, out2]

    with tile.TileContext(nc) as tc:
        with (
            tc.tile_pool(name="const", bufs=1) as constp,
            tc.tile_pool(name="xhp", bufs=1) as xhp,
            tc.tile_pool(name="zp", bufs=1) as zp,
            tc.tile_pool(name="w1p", bufs=8) as w1p,
            tc.tile_pool(name="w1qp", bufs=8) as w1qp,
            tc.tile_pool(name="w2p", bufs=2 * KD) as w2p,
            tc.tile_pool(name="outp", bufs=1) as outp,
            tc.tile_pool(name="ps_z", bufs=4, space=bass.MemorySpace.PSUM) as psz,
            tc.tile_pool(name="ps_y", bufs=3, space=bass.MemorySpace.PSUM) as psy,
        ):
            # --- input DMAs: fp8 x first (small, feeds the fp8 m1 tiles that
            # run during the DMA catch-up window), then bf16 x per k-tile ---
            xk = []
            for k in range(KD):
                t = xhp.tile([128, k_total], BF16, tag=f"xk{k}", name=f"xk{k}")
                nc.scalar.dma_start(out=t[:], in_=xh[k])
                xk.append(t)

            b1_sb = constp.tile([128, 2 * JF], F32, tag="b1")
            nc.gpsimd.dma_start(out=b1_sb[:], in_=b1c[:])
            xq = []
            for p in range(KD // 2 if nq else 0):
                t = xhp.tile([128, 2, k_total], FP8, tag=f"xq{p}", name=f"xq{p}")
                nc.gpsimd.dma_start(out=t[:], in_=xh8[p])
                xq.append(t)

            z_tiles = [
                zp.tile([128, JF, bc], z_dt, tag=f"z{bi}", name=f"z{bi}")
                for bi, (_, bc) in enumerate(blocks)
            ]
            ost = [
                outp.tile([128, KD, bc], BF16, tag=f"o{bi}", name=f"o{bi}")
                for bi, (_, bc) in enumerate(blocks)
            ]

            # --- m1: z = relu(SZ*(w1^T xhat) + SZ*b1), quantized to z_dt;
            # first nq f-tiles per half in fp8 DoubleRow, rest bf16 ---
            for bi, (csl, bc) in enumerate(blocks):
                for j in range(JF):
                    pz = psz.tile([128, bc], F32, tag="z")
                    if j >= jbf:
                        w1sb = w1qp.tile([128, KD, 128], FP8, tag="w1q")
                        nc.sync.dma_start(out=w1sb[:], in_=w1q[bi * nq + (j - jbf)])
                        for p in range(KD // 2):
                            nc.tensor.matmul(
                                pz[:], w1sb[:, 2 * p : 2 * p + 2, :],
                                xq[p][:, :, csl],
                                start=(p == 0), stop=(p == KD // 2 - 1),
                                perf_mode=DR,
                            )
                        act_scale = SZ / (SX * SW1)
                    else:
                        w1sb = w1p.tile([128, KD, 128], BF16, tag="w1")
                        nc.sync.dma_start(out=w1sb[:], in_=w1t[bi * jbf + j])
                        for k in range(KD):
                            nc.tensor.matmul(
                                pz[:], w1sb[:, k, :], xk[k][:, csl],
                                start=(k == 0), stop=(k == KD - 1),
                            )
                        act_scale = SZ if fp8_m2 else 1.0
                    nc.scalar.activation(
                        z_tiles[bi][:, j, :], pz[:], AF.Relu,
                        bias=b1_sb[:, bi * JF + j : bi * JF + j + 1],
                        scale=act_scale,
                    )

            # --- w2 tiles: queued on the sync ring behind all w1 traffic ---
            w2_tiles = []
            for s in range(2 * KD):
                w2sb = w2p.tile([128, KJ2, 128], w2_dt, tag="w2")
                nc.sync.dma_start(out=w2sb[:], in_=w2t[s])
                w2_tiles.append(w2sb)

            # --- m2: y = (z @ w2) / (SZ*SW2), bf16 out staged per block ---
            dq = 1.0 / (SZ * SW2) if fp8_m2 else 1.0
            for bi, (csl, bc) in enumerate(blocks):
                z_sb = z_tiles[bi]
                for i in range(KD):
                    w2sb = w2_tiles[bi * KD + i]
                    py = psy.tile([128, bc], F32, tag="y")
                    if fp8_m2:
                        for q in range(KJ2 // 2):
                            nc.tensor.matmul(
                                py[:], w2sb[:, 2 * q : 2 * q + 2, :],
                                z_sb[:, 2 * q : 2 * q + 2, :],
                                start=(q == 0), stop=(q == KJ2 // 2 - 1),
                                perf_mode=DR,
                            )
                    else:
                        for q in range(KJ2):
                            nc.tensor.matmul(
                                py[:], w2sb[:, q, :], z_sb[:, q, :],
                                start=(q == 0), stop=(q == KJ2 - 1),
                            )
                    nc.scalar.activation(
                        ost[bi][:, i, :], py[:], AF.Copy, bias=0.0, scale=dq
                    )
                    if i % 2 == 1:
                        nc.gpsimd.dma_start(
                            out=outs_d[bi][:, i - 1 : i + 1, :],
                            in_=ost[bi][:, i - 1 : i + 1, :],
                        )

    nc.compile()
    return nc


def _pad32(n):
    return int(max(32, ((n + 3) // 4) * 4))


def kernel(x, centroids, w1, b1, w2, b2, gamma, beta):
    x = np.ascontiguousarray(np.asarray(x, dtype=np.float32))
    centroids = np.asarray(centroids, dtype=np.float32)
    w1 = np.asarray(w1, dtype=np.float32)
    b1 = np.asarray(b1, dtype=np.float32)
    w2 = np.asarray(w2, dtype=np.float32)
    b2 = np.asarray(b2, dtype=np.float32)
    gamma = np.asarray(gamma, dtype=np.float32)
    beta = np.asarray(beta, dtype=np.float32)

    orig_shape = x.shape
    feats = x.reshape(-1, D)

    # --- host: routing + layernorm + gate (same math as the reference) ---
    aff = feats @ centroids.T
    eid = np.argmax(aff, axis=1)
    idxs = [np.nonzero(eid == e)[0] for e in range(E)]
    counts = np.array([len(ix) for ix in idxs])

    mu = feats.mean(-1, keepdims=True)
    var = feats.var(-1, keepdims=True)
    xhat = (feats - mu) / np.sqrt(var + EPS)

    # slot assignment: block 1 = halves of the 4 largest experts, block 2 =
    # halves of the 4 smallest; core c gets (ranked[c//2], half c%2) and
    # (ranked[4+c//2], half c%2).
    ranked = np.argsort(-counts, kind="stable")
    b1_cap = _pad32(counts[ranked[0]])
    b2_cap = _pad32(counts[ranked[4]])
    assert b1_cap <= 512 and b2_cap <= 512, (b1_cap, b2_cap)
    k_total = b1_cap + b2_cap

    nq = N_FP8 if USE_FP8_M2 else 0
    jbf = JF - nq
    nc = _build(b1_cap, b2_cap, USE_FP8_M2, nq)

    f8 = ml_dtypes.float8_e4m3

    def q8(a):
        return np.clip(a, -240.0, 240.0).astype(f8)

    in_maps = []
    slot_info = []  # per core: [(expert, half, offset, count), ...]
    for c in range(E):
        slots = [
            (int(ranked[c // 2]), c % 2, 0, b1_cap),
            (int(ranked[4 + c // 2]), c % 2, b1_cap, b2_cap),
        ]
        xh_full = np.zeros((D, k_total), dtype=np.float32)
        w1_tiles = np.empty((2 * jbf, 128, KD, 128), dtype=ml_dtypes.bfloat16)
        w1q_tiles = np.empty((2 * nq, 128, KD, 128), dtype=f8)
        b1_cols = np.zeros((128, 2 * JF), dtype=np.float32)
        w2_tiles = np.empty((2 * KD, 128, KJ2, 128), dtype=f8 if USE_FP8_M2 else ml_dtypes.bfloat16)
        info = []
        for bi, (e, h, off, cap) in enumerate(slots):
            n_e = counts[e]
            xh_full[:, off : off + n_e] = xhat[idxs[e]].T
            hsl = slice(h * FH, (h + 1) * FH)
            w1e = (gamma[e][:, None] * w1[e])[:, hsl]          # [D, FH]
            b1e = (b1[e] + beta[e] @ w1[e])[hsl]               # [FH]
            w1_t = w1e.reshape(KD, 128, JF, 128).transpose(2, 1, 0, 3)  # [JF,128,KD,128]
            w1q_tiles[bi * nq : (bi + 1) * nq] = q8(w1_t[jbf:] * SW1)
            w1_tiles[bi * jbf : (bi + 1) * jbf] = w1_t[:jbf].astype(ml_dtypes.bfloat16)
            b1_cols[:, bi * JF : (bi + 1) * JF] = (
                (SZ if USE_FP8_M2 else 1.0) * b1e
            ).reshape(JF, 128).T
            w2e = w2[e][hsl, :]                                # [FH, D]
            if USE_FP8_M2:
                w2q = q8(w2e * SW2)
            else:
                w2q = w2e.astype(ml_dtypes.bfloat16)
            w2_tiles[bi * KD : (bi + 1) * KD] = (
                w2q.reshape(KJ2, 128, KD, 128).transpose(2, 1, 0, 3)
            )
            info.append((e, h, off, n_e))
        xh_kt = xh_full.reshape(KD, 128, k_total)
        xh_t = np.ascontiguousarray(xh_kt).astype(ml_dtypes.bfloat16)
        im = dict(xh=xh_t, w1t=w1_tiles, b1c=b1_cols, w2t=w2_tiles)
        if nq:
            im["xh8"] = np.ascontiguousarray(
                q8(xh_kt * SX).reshape(KD // 2, 2, 128, k_total).transpose(0, 2, 1, 3)
            )
            im["w1q"] = w1q_tiles
        in_maps.append(im)
        slot_info.append(info)

    res = bass_utils.run_bass_kernel_spmd(nc, in_maps, core_ids=list(range(E)))
    kernel._last_res = res

    # --- host: sum f-half partials, gate, residual, scatter ---
    y_sum = [None] * E
    for c in range(E):
        arrs = [
            np.asarray(res.results[c]["out1"]).astype(np.float32),
            np.asarray(res.results[c]["out2"]).astype(np.float32),
        ]  # each [128, KD, B] -> [D, B]
        for bi, (e, h, off, n_e) in enumerate(slot_info[c]):
            a = arrs[bi].transpose(1, 0, 2).reshape(D, -1)
            part = a[:, :n_e].T                                # [n_e, D]
            y_sum[e] = part if y_sum[e] is None else y_sum[e] + part

    out = np.empty_like(feats)
    for e in range(E):
        ix = idxs[e]
        if len(ix) == 0:
            continue
        al = 1.0 / (1.0 + np.exp(-aff[ix, e]))[:, None]
        out[ix] = feats[ix] + al * (y_sum[e] + b2[e])
    return out.reshape(orig_shape)


# revision 16
# speedup vs baseline: 1.0511x; 1.0280x over previous
"""Expert-parallel BaseLayer MoE kernel for 8 TRN2 NeuronCores.

Strategy: all routing, layernorm, gating and the residual live on the host;
the device runs only the two FFN matmuls.  Work is split expert-parallel with
2-way f-sharding for load balance: each core owns two (expert, f-half) slots —
block 1 holds a half of one of the 4 largest experts, block 2 a half of one of
the 4 smallest — so the padded capacity is max_count + 5th_count instead of
2*max_count.  m1 runs in bf16; m2 runs in fp8-e4m3 DoubleRow (two k-tiles per
instruction at 2x rate), with the relu output quantized to fp8 by the scalar
engine and the dequant folded into the output-copy epilogue.  The host sums
the two f-half partials per expert and applies the sigmoid gate + residual.
"""

import functools
import sys

import numpy as np

for _p in ("/opt/trn_rl_repo", "/opt/pypackages"):
    if _p not in sys.path:
        sys.path.append(_p)

import ml_dtypes  # noqa: E402

import concourse.bass as bass  # noqa: E402
import concourse.mybir as mybir  # noqa: E402
import concourse.tile as tile  # noqa: E402
from concourse import bacc  # noqa: E402
from concourse import bass_utils  # noqa: E402


def _ensure_axon_hooks():
    """bass_utils' trace path imports antenv.axon_hooks, which some agent
    images lack; synthesize it (with the real ctypes NTFF hook when
    available) so tracing degrades gracefully instead of crashing."""
    try:
        import antenv.axon_hooks  # noqa: F401
        return
    except ImportError:
        pass
    import types

    import antenv

    hooks = types.ModuleType("antenv.axon_hooks")
    hooks._hook = None
    hooks.set_axon_ntff_profile_hook = lambda h: setattr(hooks, "_hook", h)
    hooks.get_axon_ntff_profile_hook = lambda: hooks._hook
    sys.modules["antenv.axon_hooks"] = hooks
    antenv.axon_hooks = hooks
    try:
        from trn_agent_boot.trn_boot import _ntff_profile_via_ctypes

        hooks._hook = _ntff_profile_via_ctypes("/opt/axon/libaxon_pjrt.so")
    except Exception:
        pass


_ensure_axon_hooks()

E = 8
D = 1024
F = 4096
FH = F // 2      # f-half per slot
KD = D // 128    # 8 k-tiles over d
JF = FH // 128   # 16 f-tiles per half (m1 output tiles)
KJ2 = FH // 128  # 16 k2-tiles per half (m2 contraction)
EPS = 1e-5

USE_FP8_M2 = True
N_FP8 = 4        # leading f-tiles per half computed in fp8 DoubleRow m1
SZ = 32.0        # z (relu output) fp8 scale
SW2 = 2048.0     # w2 fp8 scale
SX = 32.0        # xhat fp8 scale (m1 fp8 tiles)
SW1 = 1024.0     # w1 fp8 scale (m1 fp8 tiles)

F32 = mybir.dt.float32
BF16 = mybir.dt.bfloat16
FP8 = mybir.dt.float8e4
AF = mybir.ActivationFunctionType
DR = mybir.MatmulPerfMode.DoubleRow


@functools.lru_cache(maxsize=4)
def _build(b1_cap, b2_cap, fp8_m2, nq):
    k_total = b1_cap + b2_cap
    z_dt = FP8 if fp8_m2 else BF16
    w2_dt = FP8 if fp8_m2 else BF16
    jbf = JF - nq  # bf16 f-tiles per half

    nc = bacc.Bacc("TRN2", target_bir_lowering=False, debug=False, num_devices=E)

    xh = nc.declare_dram_parameter("xh", [KD, 128, k_total], BF16, isOutput=False)
    if nq:
        xh8 = nc.declare_dram_parameter(
            "xh8", [KD // 2, 128, 2, k_total], FP8, isOutput=False
        )
        w1q = nc.declare_dram_parameter(
            "w1q", [2 * nq, 128, KD, 128], FP8, isOutput=False
        )
    w1t = nc.declare_dram_parameter("w1t", [2 * jbf, 128, KD, 128], BF16, isOutput=False)
    b1c = nc.declare_dram_parameter("b1c", [128, 2 * JF], F32, isOutput=False)
    w2t = nc.declare_dram_parameter("w2t", [2 * KD, 128, KJ2, 128], w2_dt, isOutput=False)
    out1 = nc.declare_dram_parameter("out1", [128, KD, b1_cap], BF16, isOutput=True)
    out2 = nc.declare_dram_parameter("out2", [128, KD, b2_cap], BF16, isOutput=True)

    blocks = [(bass.ds(0, b1_cap), b1_cap), (bass.ds(b1_cap, b2_cap), b2_cap)]
    outs_d = [out1, out2]

    with tile.TileContext(nc) as tc:
        with (
            tc.tile_pool(name="const", bufs=1) as constp,
            tc.tile_pool(name="xhp", bufs=1) as xhp,
            tc.tile_pool(name="zp", bufs=1) as zp,
            tc.tile_pool(name="w1p", bufs=8) as w1p,
            tc.tile_pool(name="w1qp", bufs=8) as w1qp,
            tc.tile_pool(name="w2p", bufs=2 * KD) as w2p,
            tc.tile_pool(name="outp", bufs=1) as outp,
            tc.tile_pool(name="ps_z", bufs=4, space=bass.MemorySpace.PSUM) as psz,
            tc.tile_pool(name="ps_y", bufs=3, space=bass.MemorySpace.PSUM) as psy,
        ):
            # --- input DMAs: fp8 x first (small, feeds the fp8 m1 tiles that
            # run during the DMA catch-up window), then bf16 x per k-tile ---
            xk = []
            for k in range(KD):
                t = xhp.tile([128, k_total], BF16, tag=f"xk{k}", name=f"xk{k}")
                nc.scalar.dma_start(out=t[:], in_=xh[k])
                xk.append(t)

            xq = []
            for p in range(KD // 2 if nq else 0):
                t = xhp.tile([128, 2, k_total], FP8, tag=f"xq{p}", name=f"xq{p}")
                nc.scalar.dma_start(out=t[:], in_=xh8[p])
                xq.append(t)

            b1_sb = constp.tile([128, 2 * JF], F32, tag="b1")
            nc.gpsimd.dma_start(out=b1_sb[:], in_=b1c[:])

            z_tiles = [
                zp.tile([128, JF, bc], z_dt, tag=f"z{bi}", name=f"z{bi}")
                for bi, (_, bc) in enumerate(blocks)
            ]
            ost = [
                outp.tile([128, KD, bc], BF16, tag=f"o{bi}", name=f"o{bi}")
                for bi, (_, bc) in enumerate(blocks)
            ]

            # --- m1: z = relu(SZ*(w1^T xhat) + SZ*b1), quantized to z_dt;
            # first nq f-tiles per half in fp8 DoubleRow, rest bf16 ---
            for bi, (csl, bc) in enumerate(blocks):
                for j in range(JF):
                    pz = psz.tile([128, bc], F32, tag="z")
                    if j >= jbf:
                        w1sb = w1qp.tile([128, KD, 128], FP8, tag="w1q")
                        nc.sync.dma_start(out=w1sb[:], in_=w1q[bi * nq + (j - jbf)])
                        for p in range(KD // 2):
                            nc.tensor.matmul(
                                pz[:], w1sb[:, 2 * p : 2 * p + 2, :],
                                xq[p][:, :, csl],
                                start=(p == 0), stop=(p == KD // 2 - 1),
                                perf_mode=DR,
                            )
                        act_scale = SZ / (SX * SW1)
                    else:
                        w1sb = w1p.tile([128, KD, 128], BF16, tag="w1")
                        nc.sync.dma_start(out=w1sb[:], in_=w1t[bi * jbf + j])
                        for k in range(KD):
                            nc.tensor.matmul(
                                pz[:], w1sb[:, k, :], xk[k][:, csl],
                                start=(k == 0), stop=(k == KD - 1),
                            )
                        act_scale = SZ if fp8_m2 else 1.0
                    nc.scalar.activation(
                        z_tiles[bi][:, j, :], pz[:], AF.Relu,
                        bias=b1_sb[:, bi * JF + j : bi * JF + j + 1],
                        scale=act_scale,
                    )

            # --- w2 tiles: queued on the sync ring behind all w1 traffic ---
            w2_tiles = []
            for s in range(2 * KD):
                w2sb = w2p.tile([128, KJ2, 128], w2_dt, tag="w2")
                nc.sync.dma_start(out=w2sb[:], in_=w2t[s])
                w2_tiles.append(w2sb)

            # --- m2: y = (z @ w2) / (SZ*SW2), bf16 out staged per block ---
            dq = 1.0 / (SZ * SW2) if fp8_m2 else 1.0
            for bi, (csl, bc) in enumerate(blocks):
                z_sb = z_tiles[bi]
                for i in range(KD):
                    w2sb = w2_tiles[bi * KD + i]
                    py = psy.tile([128, bc], F32, tag="y")
                    if fp8_m2:
                        for q in range(KJ2 // 2):
                            nc.tensor.matmul(
                                py[:], w2sb[:, 2 * q : 2 * q + 2, :],
                                z_sb[:, 2 * q : 2 * q + 2, :],
                                start=(q == 0), stop=(q == KJ2 // 2 - 1),
                                perf_mode=DR,
                            )
                    else:
                        for q in range(KJ2):
                            nc.tensor.matmul(
                                py[:], w2sb[:, q, :], z_sb[:, q, :],
                                start=(q == 0), stop=(q == KJ2 - 1),
                            )
                    nc.scalar.activation(
                        ost[bi][:, i, :], py[:], AF.Copy, bias=0.0, scale=dq
                    )
                    if i % 2 == 1:
                        nc.gpsimd.dma_start(
                            out=outs_d[bi][:, i - 1 : i + 1, :],
                            in_=ost[bi][:, i - 1 : i + 1, :],
                        )

    nc.compile()
    return nc


def _pad32(n):
    return int(max(32, ((n + 3) // 4) * 4))


def kernel(x, centroids, w1, b1, w2, b2, gamma, beta):
    x = np.ascontiguousarray(np.asarray(x, dtype=np.float32))
    centroids = np.asarray(centroids, dtype=np.float32)
    w1 = np.asarray(w1, dtype=np.float32)
    b1 = np.asarray(b1, dtype=np.float32)
    w2 = np.asarray(w2, dtype=np.float32)
    b2 = np.asarray(b2, dtype=np.float32)
    gamma = np.asarray(gamma, dtype=np.float32)
    beta = np.asarray(beta, dtype=np.float32)

    orig_shape = x.shape
    feats = x.reshape(-1, D)

    # --- host: routing + layernorm + gate (same math as the reference) ---
    aff = feats @ centroids.T
    eid = np.argmax(aff, axis=1)
    idxs = [np.nonzero(eid == e)[0] for e in range(E)]
    counts = np.array([len(ix) for ix in idxs])

    mu = feats.mean(-1, keepdims=True)
    var = feats.var(-1, keepdims=True)
    xhat = (feats - mu) / np.sqrt(var + EPS)

    # slot assignment: block 1 = halves of the 4 largest experts, block 2 =
    # halves of the 4 smallest; core c gets (ranked[c//2], half c%2) and
    # (ranked[4+c//2], half c%2).
    ranked = np.argsort(-counts, kind="stable")
    b1_cap = _pad32(counts[ranked[0]])
    b2_cap = _pad32(counts[ranked[4]])
    assert b1_cap <= 512 and b2_cap <= 512, (b1_cap, b2_cap)
    k_total = b1_cap + b2_cap

    nq = N_FP8 if USE_FP8_M2 else 0
    jbf = JF - nq
    nc = _build(b1_cap, b2_cap, USE_FP8_M2, nq)

    f8 = ml_dtypes.float8_e4m3

    def q8(a):
        return np.clip(a, -240.0, 240.0).astype(f8)

    in_maps = []
    slot_info = []  # per core: [(expert, half, offset, count), ...]
    for c in range(E):
        slots = [
            (int(ranked[c // 2]), c % 2, 0, b1_cap),
            (int(ranked[4 + c // 2]), c % 2, b1_cap, b2_cap),
        ]
        xh_full = np.zeros((D, k_total), dtype=np.float32)
        w1_tiles = np.empty((2 * jbf, 128, KD, 128), dtype=ml_dtypes.bfloat16)
        w1q_tiles = np.empty((2 * nq, 128, KD, 128), dtype=f8)
        b1_cols = np.zeros((128, 2 * JF), dtype=np.float32)
        w2_tiles = np.empty((2 * KD, 128, KJ2, 128), dtype=f8 if USE_FP8_M2 else ml_dtypes.bfloat16)
        info = []
        for bi, (e, h, off, cap) in enumerate(slots):
            n_e = counts[e]
            xh_full[:, off : off + n_e] = xhat[idxs[e]].T
            hsl = slice(h * FH, (h + 1) * FH)
            w1e = (gamma[e][:, None] * w1[e])[:, hsl]          # [D, FH]
            b1e = (b1[e] + beta[e] @ w1[e])[hsl]               # [FH]
            w1_t = w1e.reshape(KD, 128, JF, 128).transpose(2, 1, 0, 3)  # [JF,128,KD,128]
            w1q_tiles[bi * nq : (bi + 1) * nq] = q8(w1_t[jbf:] * SW1)
            w1_tiles[bi * jbf : (bi + 1) * jbf] = w1_t[:jbf].astype(ml_dtypes.bfloat16)
            b1_cols[:, bi * JF : (bi + 1) * JF] = (
                (SZ if USE_FP8_M2 else 1.0) * b1e
            ).reshape(JF, 128).T
            w2e = w2[e][hsl, :]                                # [FH, D]
            if USE_FP8_M2:
                w2q = q8(w2e * SW2)
            else:
                w2q = w2e.astype(ml_dtypes.bfloat16)
            w2_tiles[bi * KD : (bi + 1) * KD] = (
                w2q.reshape(KJ2, 128, KD, 128).transpose(2, 1, 0, 3)
            )
            info.append((e, h, off, n_e))
        xh_kt = xh_full.reshape(KD, 128, k_total)
        xh_t = np.ascontiguousarray(xh_kt).astype(ml_dtypes.bfloat16)
        im = dict(xh=xh_t, w1t=w1_tiles, b1c=b1_cols, w2t=w2_tiles)
        if nq:
            im["xh8"] = np.ascontiguousarray(
                q8(xh_kt * SX).reshape(KD // 2, 2, 128, k_total).transpose(0, 2, 1, 3)
            )
            im["w1q"] = w1q_tiles
        in_maps.append(im)
        slot_info.append(info)

    res = bass_utils.run_bass_kernel_spmd(nc, in_maps, core_ids=list(range(E)))
    kernel._last_res = res

    # --- host: sum f-half partials, gate, residual, scatter ---
    y_sum = [None] * E
    for c in range(E):
        arrs = [
            np.asarray(res.results[c]["out1"]).astype(np.float32),
            np.asarray(res.results[c]["out2"]).astype(np.float32),
        ]  # each [128, KD, B] -> [D, B]
        for bi, (e, h, off, n_e) in enumerate(slot_info[c]):
            a = arrs[bi].transpose(1, 0, 2).reshape(D, -1)
            part = a[:, :n_e].T                                # [n_e, D]
            y_sum[e] = part if y_sum[e] is None else y_sum[e] + part

    out = np.empty_like(feats)
    for e in range(E):
        ix = idxs[e]
        if len(ix) == 0:
            continue
        al = 1.0 / (1.0 + np.exp(-aff[ix, e]))[:, None]
        out[ix] = feats[ix] + al * (y_sum[e] + b2[e])
    return out.reshape(orig_shape)


# revision 17
# speedup vs baseline: 1.0601x; 1.0085x over previous
"""Expert-parallel BaseLayer MoE kernel for 8 TRN2 NeuronCores.

Strategy: all routing, layernorm, gating and the residual live on the host;
the device runs only the two FFN matmuls.  Work is split expert-parallel with
2-way f-sharding for load balance: each core owns two (expert, f-half) slots —
block 1 holds a half of one of the 4 largest experts, block 2 a half of one of
the 4 smallest — so the padded capacity is max_count + 5th_count instead of
2*max_count.  m1 runs in bf16; m2 runs in fp8-e4m3 DoubleRow (two k-tiles per
instruction at 2x rate), with the relu output quantized to fp8 by the scalar
engine and the dequant folded into the output-copy epilogue.  The host sums
the two f-half partials per expert and applies the sigmoid gate + residual.
"""

import functools
import sys

import numpy as np

for _p in ("/opt/trn_rl_repo", "/opt/pypackages"):
    if _p not in sys.path:
        sys.path.append(_p)

import ml_dtypes  # noqa: E402

import concourse.bass as bass  # noqa: E402
import concourse.mybir as mybir  # noqa: E402
import concourse.tile as tile  # noqa: E402
from concourse import bacc  # noqa: E402
from concourse import bass_utils  # noqa: E402


def _ensure_axon_hooks():
    """bass_utils' trace path imports antenv.axon_hooks, which some agent
    images lack; synthesize it (with the real ctypes NTFF hook when
    available) so tracing degrades gracefully instead of crashing."""
    try:
        import antenv.axon_hooks  # noqa: F401
        return
    except ImportError:
        pass
    import types

    import antenv

    hooks = types.ModuleType("antenv.axon_hooks")
    hooks._hook = None
    hooks.set_axon_ntff_profile_hook = lambda h: setattr(hooks, "_hook", h)
    hooks.get_axon_ntff_profile_hook = lambda: hooks._hook
    sys.modules["antenv.axon_hooks"] = hooks
    antenv.axon_hooks = hooks
    try:
        from trn_agent_boot.trn_boot import _ntff_profile_via_ctypes

        hooks._hook = _ntff_profile_via_ctypes("/opt/axon/libaxon_pjrt.so")
    except Exception:
        pass


_ensure_axon_hooks()

E = 8
D = 1024
F = 4096
FH = F // 2      # f-half per slot
KD = D // 128    # 8 k-tiles over d
JF = FH // 128   # 16 f-tiles per half (m1 output tiles)
KJ2 = FH // 128  # 16 k2-tiles per half (m2 contraction)
EPS = 1e-5

USE_FP8_M2 = True
N_FP8 = 4        # leading f-tiles per half computed in fp8 DoubleRow m1
SZ = 32.0        # z (relu output) fp8 scale
SW2 = 2048.0     # w2 fp8 scale
SX = 32.0        # xhat fp8 scale (m1 fp8 tiles)
SW1 = 1024.0     # w1 fp8 scale (m1 fp8 tiles)

F32 = mybir.dt.float32
BF16 = mybir.dt.bfloat16
FP8 = mybir.dt.float8e4
AF = mybir.ActivationFunctionType
DR = mybir.MatmulPerfMode.DoubleRow


@functools.lru_cache(maxsize=4)
def _build(b1_cap, b2_cap, fp8_m2, nq):
    k_total = b1_cap + b2_cap
    z_dt = FP8 if fp8_m2 else BF16
    w2_dt = FP8 if fp8_m2 else BF16
    jbf = JF - nq  # bf16 f-tiles per half

    nc = bacc.Bacc("TRN2", target_bir_lowering=False, debug=False, num_devices=E)

    xh = nc.declare_dram_parameter("xh", [KD, 128, k_total], BF16, isOutput=False)
    if nq:
        w1q = nc.declare_dram_parameter(
            "w1q", [2 * nq, 128, KD, 128], FP8, isOutput=False
        )
    w1t = nc.declare_dram_parameter("w1t", [2 * jbf, 128, KD, 128], BF16, isOutput=False)
    b1c = nc.declare_dram_parameter("b1c", [128, 2 * JF], F32, isOutput=False)
    w2t = nc.declare_dram_parameter("w2t", [2 * KD, 128, KJ2, 128], w2_dt, isOutput=False)
    out1 = nc.declare_dram_parameter("out1", [128, KD, b1_cap], BF16, isOutput=True)
    out2 = nc.declare_dram_parameter("out2", [128, KD, b2_cap], BF16, isOutput=True)

    blocks = [(bass.ds(0, b1_cap), b1_cap), (bass.ds(b1_cap, b2_cap), b2_cap)]
    outs_d = [out1, out2]

    with tile.TileContext(nc) as tc:
        with (
            tc.tile_pool(name="const", bufs=1) as constp,
            tc.tile_pool(name="xhp", bufs=1) as xhp,
            tc.tile_pool(name="zp", bufs=1) as zp,
            tc.tile_pool(name="w1p", bufs=12) as w1p,
            tc.tile_pool(name="w1qp", bufs=8) as w1qp,
            tc.tile_pool(name="w2p", bufs=2 * KD) as w2p,
            tc.tile_pool(name="outp", bufs=1) as outp,
            tc.tile_pool(name="ps_z", bufs=4, space=bass.MemorySpace.PSUM) as psz,
            tc.tile_pool(name="ps_y", bufs=3, space=bass.MemorySpace.PSUM) as psy,
        ):
            # --- input DMAs: fp8 x first (small, feeds the fp8 m1 tiles that
            # run during the DMA catch-up window), then bf16 x per k-tile ---
            xk = []
            for k in range(KD):
                t = xhp.tile([128, k_total], BF16, tag=f"xk{k}", name=f"xk{k}")
                nc.scalar.dma_start(out=t[:], in_=xh[k])
                xk.append(t)

            xq = []
            for p in range(KD // 2 if nq else 0):
                t = xhp.tile([128, 2, k_total], FP8, tag=f"xq{p}", name=f"xq{p}")
                for i in range(2):
                    nc.vector.tensor_scalar_mul(t[:, i, :], xk[2 * p + i][:], SX)
                xq.append(t)

            b1_sb = constp.tile([128, 2 * JF], F32, tag="b1")
            nc.gpsimd.dma_start(out=b1_sb[:], in_=b1c[:])

            z_tiles = [
                zp.tile([128, JF, bc], z_dt, tag=f"z{bi}", name=f"z{bi}")
                for bi, (_, bc) in enumerate(blocks)
            ]
            ost = [
                outp.tile([128, KD, bc], BF16, tag=f"o{bi}", name=f"o{bi}")
                for bi, (_, bc) in enumerate(blocks)
            ]

            # --- m1: z = relu(SZ*(w1^T xhat) + SZ*b1), quantized to z_dt;
            # first nq f-tiles per half in fp8 DoubleRow, rest bf16 ---
            for bi, (csl, bc) in enumerate(blocks):
                for j in range(JF):
                    pz = psz.tile([128, bc], F32, tag="z")
                    if j >= jbf:
                        w1sb = w1qp.tile([128, KD, 128], FP8, tag="w1q")
                        nc.sync.dma_start(out=w1sb[:], in_=w1q[bi * nq + (j - jbf)])
                        for p in range(KD // 2):
                            nc.tensor.matmul(
                                pz[:], w1sb[:, 2 * p : 2 * p + 2, :],
                                xq[p][:, :, csl],
                                start=(p == 0), stop=(p == KD // 2 - 1),
                                perf_mode=DR,
                            )
                        act_scale = SZ / (SX * SW1)
                    else:
                        w1sb = w1p.tile([128, KD, 128], BF16, tag="w1")
                        nc.sync.dma_start(out=w1sb[:], in_=w1t[bi * jbf + j])
                        for k in range(KD):
                            nc.tensor.matmul(
                                pz[:], w1sb[:, k, :], xk[k][:, csl],
                                start=(k == 0), stop=(k == KD - 1),
                            )
                        act_scale = SZ if fp8_m2 else 1.0
                    nc.scalar.activation(
                        z_tiles[bi][:, j, :], pz[:], AF.Relu,
                        bias=b1_sb[:, bi * JF + j : bi * JF + j + 1],
                        scale=act_scale,
                    )

            # --- w2 tiles: queued on the sync ring behind all w1 traffic ---
            w2_tiles = []
            for s in range(2 * KD):
                w2sb = w2p.tile([128, KJ2, 128], w2_dt, tag="w2")
                nc.sync.dma_start(out=w2sb[:], in_=w2t[s])
                w2_tiles.append(w2sb)

            # --- m2: y = (z @ w2) / (SZ*SW2), bf16 out staged per block ---
            dq = 1.0 / (SZ * SW2) if fp8_m2 else 1.0
            for bi, (csl, bc) in enumerate(blocks):
                z_sb = z_tiles[bi]
                for i in range(KD):
                    w2sb = w2_tiles[bi * KD + i]
                    py = psy.tile([128, bc], F32, tag="y")
                    if fp8_m2:
                        for q in range(KJ2 // 2):
                            nc.tensor.matmul(
                                py[:], w2sb[:, 2 * q : 2 * q + 2, :],
                                z_sb[:, 2 * q : 2 * q + 2, :],
                                start=(q == 0), stop=(q == KJ2 // 2 - 1),
                                perf_mode=DR,
                            )
                    else:
                        for q in range(KJ2):
                            nc.tensor.matmul(
                                py[:], w2sb[:, q, :], z_sb[:, q, :],
                                start=(q == 0), stop=(q == KJ2 - 1),
                            )
                    nc.scalar.activation(
                        ost[bi][:, i, :], py[:], AF.Copy, bias=0.0, scale=dq
                    )
                    if i % 2 == 1:
                        nc.gpsimd.dma_start(
                            out=outs_d[bi][:, i - 1 : i + 1, :],
                            in_=ost[bi][:, i - 1 : i + 1, :],
                        )

    nc.compile()
    return nc


def _pad32(n):
    return int(max(32, ((n + 3) // 4) * 4))


def kernel(x, centroids, w1, b1, w2, b2, gamma, beta):
    x = np.ascontiguousarray(np.asarray(x, dtype=np.float32))
    centroids = np.asarray(centroids, dtype=np.float32)
    w1 = np.asarray(w1, dtype=np.float32)
    b1 = np.asarray(b1, dtype=np.float32)
    w2 = np.asarray(w2, dtype=np.float32)
    b2 = np.asarray(b2, dtype=np.float32)
    gamma = np.asarray(gamma, dtype=np.float32)
    beta = np.asarray(beta, dtype=np.float32)

    orig_shape = x.shape
    feats = x.reshape(-1, D)

    # --- host: routing + layernorm + gate (same math as the reference) ---
    aff = feats @ centroids.T
    eid = np.argmax(aff, axis=1)
    idxs = [np.nonzero(eid == e)[0] for e in range(E)]
    counts = np.array([len(ix) for ix in idxs])

    mu = feats.mean(-1, keepdims=True)
    var = feats.var(-1, keepdims=True)
    xhat = (feats - mu) / np.sqrt(var + EPS)

    # slot assignment: block 1 = halves of the 4 largest experts, block 2 =
    # halves of the 4 smallest; core c gets (ranked[c//2], half c%2) and
    # (ranked[4+c//2], half c%2).
    ranked = np.argsort(-counts, kind="stable")
    b1_cap = _pad32(counts[ranked[0]])
    b2_cap = _pad32(counts[ranked[4]])
    assert b1_cap <= 512 and b2_cap <= 512, (b1_cap, b2_cap)
    k_total = b1_cap + b2_cap

    nq = N_FP8 if USE_FP8_M2 else 0
    jbf = JF - nq
    nc = _build(b1_cap, b2_cap, USE_FP8_M2, nq)

    f8 = ml_dtypes.float8_e4m3

    def q8(a):
        return np.clip(a, -240.0, 240.0).astype(f8)

    in_maps = []
    slot_info = []  # per core: [(expert, half, offset, count), ...]
    for c in range(E):
        slots = [
            (int(ranked[c // 2]), c % 2, 0, b1_cap),
            (int(ranked[4 + c // 2]), c % 2, b1_cap, b2_cap),
        ]
        xh_full = np.zeros((D, k_total), dtype=np.float32)
        w1_tiles = np.empty((2 * jbf, 128, KD, 128), dtype=ml_dtypes.bfloat16)
        w1q_tiles = np.empty((2 * nq, 128, KD, 128), dtype=f8)
        b1_cols = np.zeros((128, 2 * JF), dtype=np.float32)
        w2_tiles = np.empty((2 * KD, 128, KJ2, 128), dtype=f8 if USE_FP8_M2 else ml_dtypes.bfloat16)
        info = []
        for bi, (e, h, off, cap) in enumerate(slots):
            n_e = counts[e]
            xh_full[:, off : off + n_e] = xhat[idxs[e]].T
            hsl = slice(h * FH, (h + 1) * FH)
            w1e = (gamma[e][:, None] * w1[e])[:, hsl]          # [D, FH]
            b1e = (b1[e] + beta[e] @ w1[e])[hsl]               # [FH]
            w1_t = w1e.reshape(KD, 128, JF, 128).transpose(2, 1, 0, 3)  # [JF,128,KD,128]
            w1q_tiles[bi * nq : (bi + 1) * nq] = q8(w1_t[jbf:] * SW1)
            w1_tiles[bi * jbf : (bi + 1) * jbf] = w1_t[:jbf].astype(ml_dtypes.bfloat16)
            b1_cols[:, bi * JF : (bi + 1) * JF] = (
                (SZ if USE_FP8_M2 else 1.0) * b1e
            ).reshape(JF, 128).T
            w2e = w2[e][hsl, :]                                # [FH, D]
            if USE_FP8_M2:
                w2q = q8(w2e * SW2)
            else:
                w2q = w2e.astype(ml_dtypes.bfloat16)
            w2_tiles[bi * KD : (bi + 1) * KD] = (
                w2q.reshape(KJ2, 128, KD, 128).transpose(2, 1, 0, 3)
            )
            info.append((e, h, off, n_e))
        xh_kt = xh_full.reshape(KD, 128, k_total)
        xh_t = np.ascontiguousarray(xh_kt).astype(ml_dtypes.bfloat16)
        im = dict(xh=xh_t, w1t=w1_tiles, b1c=b1_cols, w2t=w2_tiles)
        if nq:
            im["w1q"] = w1q_tiles
        in_maps.append(im)
        slot_info.append(info)

    res = bass_utils.run_bass_kernel_spmd(nc, in_maps, core_ids=list(range(E)))
    kernel._last_res = res

    # --- host: sum f-half partials, gate, residual, scatter ---
    y_sum = [None] * E
    for c in range(E):
        arrs = [
            np.asarray(res.results[c]["out1"]).astype(np.float32),
            np.asarray(res.results[c]["out2"]).astype(np.float32),
        ]  # each [128, KD, B] -> [D, B]
        for bi, (e, h, off, n_e) in enumerate(slot_info[c]):
            a = arrs[bi].transpose(1, 0, 2).reshape(D, -1)
            part = a[:, :n_e].T                                # [n_e, D]
            y_sum[e] = part if y_sum[e] is None else y_sum[e] + part

    out = np.empty_like(feats)
    for e in range(E):
        ix = idxs[e]
        if len(ix) == 0:
            continue
        al = 1.0 / (1.0 + np.exp(-aff[ix, e]))[:, None]
        out[ix] = feats[ix] + al * (y_sum[e] + b2[e])
    return out.reshape(orig_shape)
